# revision 1
# baseline (speedup 1.0000x reference)
"""Trainium2 Bass kernel for nn_DistanceEdgeSelfCond.

Computes, for inputs pred_coords [8,512,3], mask [8,512], W [64,32], b [64]:
    d[i,j]   = ||x_i - x_j||                        (pairwise distances)
    rbf      = exp(coeff * (d - o_k)^2)             (gaussian smearing, K=32)
    edge     = rbf @ W.T + b                        ([B,512,512,64])
    out      = edge * (mask_i * mask_j)[...,None]

Sharding: data-parallel over B — one batch per NeuronCore (8 cores).

Device strategy (per core, one batch element):
  * The output is symmetric in (i,j), so the device computes only the
    upper block-triangle (8-row blocks); the host mirrors the rest.
    33 uniform iterations cover it: row-block 0 alone (64 j-chunks),
    pairs (a, 64-a) for a=1..31 (64 j-chunks combined), row-block 32
    alone (32 chunks).
  * Per 4-row half-block, ONE fp16 matmul (contraction over 120
    partitions holding {hi/lo splits of coeff*d^2 and d} x 4 rows x 5
    terms for 6 half-blocks) produces arg = coeff*(d^2 - 2*o_k*d) in
    PSUM at [128=(i_sub,k), j] layout; coeff*o_k^2 rides the ACT Exp
    bias.  fp16 products are exact in the f32 PSUM, and hi/lo splits
    carry f32-grade precision through the fp16 operands.
  * ACT Exp (with per-partition bias) -> rbf fp16, laid out as the
    edge-matmul lhsT.
  * Edge matmul vs block-diagonal W.T (fp16): 8 matmuls of 256 cols
    per iteration; out partitions = (j-octet, i-half), cols = (i_sub,d).
  * PSUM evacuation (f32 -> fp16) rotates across DVE / Pool / ACT.
  * Output written to HBM as fp16 (halves DMA-write traffic); host
    upcasts, adds the bias b, applies the mask, and mirrors the lower
    triangle.

Walrus's PE LDWEIGHTS struct carries at most ONE sync wait, so a
post-pass moves excess waits onto InstNoOp in the same engine stream.
"""

import sys

import numpy as np

for _p in ("/opt/trn_rl_repo", "/root/.axon_site/_ro/trn_rl_repo"):
    if _p not in sys.path:
        sys.path.append(_p)

B = 8
N = 512
K = 32
D = 64
CUTOFF = 10.0

# column offsets inside the merged constant tensor [128, CW] (f32)
C_LG = 0          # rows 0:5, cols 0:512
C_RG = 512        # rows 0:5, cols 512:1024
C_DM = 1024       # [128, 2048] diag-zero mask * coeff, per 128-i chunk
C_SEL = 3072      # rows 0:120, 6 x 128 cols  (sel6 stationary variants)
C_WC = 3840       # [128, 256]  block-diagonal W.T
C_OB = 4096       # [128, 1]    coeff * o_k^2 per partition (k = p % 32)
CW = 4097

NG = 22           # gather groups of 6 half-blocks (last partial: 2)

_CACHE = {}
TRACE = False  # set True (e.g. from test.py) to capture an NTFF profile


def _fix_waits(nc, mybir):
    """Enforce <=1 embedded sync wait on compute-engine instructions."""
    limited = {
        mybir.EngineType.PE,
        mybir.EngineType.DVE,
        mybir.EngineType.Activation,
        mybir.EngineType.SP,
        mybir.EngineType.Pool,
    }
    for blk in nc.m.functions[0].blocks:
        insts = blk.instructions
        i = 0
        while i < len(insts):
            inst = insts[i]
            si = inst.sync_info
            if (
                inst.engine in limited
                and si is not None
                and si.on_wait
                and len(si.on_wait) > 1
            ):
                waits = list(si.on_wait)
                excess, keep = waits[:-1], waits[-1:]
                for w in excess:
                    nop = mybir.InstNoOp(
                        name=nc.get_next_instruction_name(),
                        sync_info=mybir.SyncInfo(on_wait=[w], on_update=[]),
                        bass_nofuse=True,
                        engine=inst.engine,
                    )
                    nc.register_instruction(nop)
                    insts.insert(i, nop)
                    i += 1
                si.on_wait = keep
            i += 1


def _iters():
    """(A, B) row-block pairs: 33 iterations covering the block triangle."""
    its = [(0, None)]
    its += [(a, 64 - a) for a in range(1, 32)]
    its.append((32, None))
    return its


def _build_program():
    import concourse.bass as bass
    import concourse.tile as tile
    from concourse import mybir

    f32 = mybir.dt.float32
    f16 = mybir.dt.float16
    AF = mybir.ActivationFunctionType

    o = np.linspace(0.0, CUTOFF, K)
    coeff = float(-0.5 / (o[1] - o[0]) ** 2)

    nc = bass.Bass("TRN2", target_bir_lowering=False, debug=False)

    ct_d = nc.dram_tensor("ct", [128, CW], f32, kind="ExternalInput")
    out_d = nc.dram_tensor("out", [N, N, D], f16, kind="ExternalOutput")
    out3 = out_d.ap()

    with tile.TileContext(nc) as tc:
        with (
            tc.tile_pool(name="consts", bufs=1) as consts,
            tc.tile_pool(name="dstore", bufs=1) as dstore,
            tc.tile_pool(name="work", bufs=2) as work,
            tc.tile_pool(name="rbfp", bufs=3) as rbfp,
            tc.tile_pool(name="stpool", bufs=4) as stpool,
            tc.tile_pool(name="psA", bufs=2, space=bass.MemorySpace.PSUM) as psA,
            tc.tile_pool(name="psB", bufs=4, space=bass.MemorySpace.PSUM) as psB,
        ):
            ct_s = consts.tile([128, CW], f32, tag="ct")
            ap = ct_d.ap()
            # phase-1 deps first, bulk behind
            nc.sync.dma_start(ct_s[0:5, 0:1024], ap[0:5, 0:1024])
            nc.sync.dma_start(ct_s[:, C_DM : C_DM + 2048], ap[:, C_DM : C_DM + 2048])
            nc.sync.dma_start(
                ct_s[0:120, C_SEL : C_SEL + 768], ap[0:120, C_SEL : C_SEL + 768]
            )
            nc.sync.dma_start(
                ct_s[:, C_WC : C_WC + 257], ap[:, C_WC : C_WC + 257]
            )

            ob_s = ct_s[:, C_OB : C_OB + 1]

            # fp16 casts of matmul constants
            sel6f = consts.tile([120, 6 * 128], f16, tag="sel6f")
            nc.vector.tensor_copy(sel6f[:], ct_s[0:120, C_SEL : C_SEL + 768])
            wcf = consts.tile([128, 256], f16, tag="wcf")
            nc.vector.tensor_copy(wcf[:], ct_s[:, C_WC : C_WC + 256])

            # X5 [128, (t=5, q=4, j=512)]: per-i-row fp16 splits
            #   t0 = hi(coeff*d^2), t1 = lo, t2 = t3 = hi(d), t4 = lo(d)
            X5 = dstore.tile([128, 5 * 2048], f16, tag="X5")

            for q in range(4):
                g_ps = psB.tile([128, N], f32, tag="eps")
                nc.tensor.matmul(
                    g_ps[:],
                    ct_s[0:5, C_LG + q * 128 : C_LG + (q + 1) * 128],
                    ct_s[0:5, C_RG : C_RG + N],
                )
                draw = work.tile([128, N], f32, tag="draw")
                # a1 = relu(d^2) * coeff, diagonal zeroed (dm carries coeff)
                nc.vector.scalar_tensor_tensor(
                    draw[:],
                    g_ps[:],
                    0.0,
                    ct_s[:, C_DM + q * N : C_DM + (q + 1) * N],
                    mybir.AluOpType.max,
                    mybir.AluOpType.mult,
                )
                qs = q * N
                nc.gpsimd.tensor_copy(X5[:, 0 * 2048 + qs : 0 * 2048 + qs + N], draw[:])
                nc.gpsimd.tensor_sub(
                    X5[:, 1 * 2048 + qs : 1 * 2048 + qs + N],
                    draw[:],
                    X5[:, 0 * 2048 + qs : 0 * 2048 + qs + N],
                )
                dfull = work.tile([128, N], f32, tag="dfull")
                nc.scalar.activation(
                    dfull[:], draw[:], AF.Sqrt, scale=float(1.0 / coeff)
                )
                nc.gpsimd.tensor_copy(X5[:, 2 * 2048 + qs : 2 * 2048 + qs + N], dfull[:])
                nc.gpsimd.tensor_copy(X5[:, 3 * 2048 + qs : 3 * 2048 + qs + N], dfull[:])
                nc.gpsimd.tensor_sub(
                    X5[:, 4 * 2048 + qs : 4 * 2048 + qs + N],
                    dfull[:],
                    X5[:, 2 * 2048 + qs : 2 * 2048 + qs + N],
                )

            # movb [120, NG*512] fp16: per 6-half-block group G, partition
            # (m*4 + i_sub)*5 + t holds term t of i-row 24G + 4m + i_sub.
            movb = dstore.tile([120, NG * 512], f16, tag="movb")
            # last group holds only 2 half-blocks (40 rows); zero its chunk
            # first (gather overwrites rows 0:40) so inactive-row garbage
            # can't turn 0-cell products into NaN
            nc.vector.memset(movb[0:120, 512 * (NG - 1) : 512 * NG], 0.0)

            def emit_gather(G):
                nmem = min(6, 128 - 6 * G)  # half-blocks in this group
                r0 = 24 * G  # first global i-row
                r1 = r0 + 4 * nmem
                # split into per-q runs (q = i//128)
                s = r0
                while s < r1:
                    q = s // 128
                    e = min(r1, (q + 1) * 128)
                    p0, cnt = s % 128, e - s
                    src = X5[p0 : p0 + cnt, :].rearrange(
                        "r (t q j) -> r t q j", t=5, q=4
                    )[:, :, q, :]
                    # dst partition (r*5 + t) is r-major: flat [5*cnt, 512]
                    # enumerates (r, t, j) in the same order as src
                    dst = movb[
                        5 * (s - r0) : 5 * (s - r0) + 5 * cnt,
                        512 * G : 512 * (G + 1),
                    ]
                    nc.sync.dma_start(dst, src)
                    s = e

            # gathers ordered by first use (iterations walk both ends)
            order, seen = [], set()
            for (A, Bb) in _iters():
                hbs = [2 * A, 2 * A + 1] + ([2 * Bb, 2 * Bb + 1] if Bb else [])
                for hb in hbs:
                    G = hb // 6
                    if G not in seen:
                        seen.add(G)
                        order.append(G)
            for G in order:
                emit_gather(G)

            its = _iters()
            diff_tiles = {}

            def emit_bcast(t):
                # diff col = p*16 + h*8 + e (p = j-octet in the A|B concat,
                # h = i-half, e = j%8): slot s = 2p+h is the edge-matmul out
                # partition, so A and B land on contiguous partition runs
                # while each DMA dst run is a full (e d) = 1 KiB.
                A, Bb = its[t]
                jcA = 512 - 8 * A
                diff = psA.tile([128, 1024], f32, tag="diff")
                dv = diff.rearrange("p (u h e) -> p h u e", u=64, h=2, e=8)
                for h in (0, 1):
                    hbA = 2 * A + h
                    GA, mA = hbA // 6, hbA % 6
                    selA = sel6f[:, mA * 128 : (mA + 1) * 128]
                    movA = movb[:, 512 * GA + 8 * A : 512 * GA + 512]
                    # a matmul whose strided PSUM out crosses the 2 KiB bank
                    # boundary gets its second-bank tail clobbered by the
                    # next matmul in that bank — split at octet 32
                    nA = jcA // 8
                    nc.tensor.matmul(
                        dv[:, h, 0 : min(nA, 32)], selA, movA[:, 0 : min(jcA, 256)]
                    )
                    if nA > 32:
                        nc.tensor.matmul(
                            dv[:, h, 32:nA], selA, movA[:, 256:jcA]
                        )
                    if Bb is not None:
                        hbB = 2 * Bb + h
                        GB, mB = hbB // 6, hbB % 6
                        nc.tensor.matmul(
                            dv[:, h, nA:64],
                            sel6f[:, mB * 128 : (mB + 1) * 128],
                            movb[:, 512 * GB + 8 * Bb : 512 * GB + 512],
                        )
                diff_tiles[t] = diff

            LOOKAHEAD = 1
            for t in range(LOOKAHEAD):
                emit_bcast(t)

            for t in range(len(its)):
                A, Bb = its[t]
                jcA = 512 - 8 * A
                nsA = jcA // 4           # A slots (4 j-pixels each)
                nsB = (512 - 8 * Bb) // 4 if Bb is not None else 0
                npart = nsA + nsB        # 128 except final half iteration

                if t + LOOKAHEAD < len(its):
                    emit_bcast(t + LOOKAHEAD)
                diff = diff_tiles.pop(t)

                rbf = rbfp.tile([128, 1024], f16, tag="rbf")
                if npart == 128:
                    nc.scalar.activation(rbf[:], diff[:], AF.Exp, bias=ob_s)
                else:
                    # half iteration: used cols are the contiguous [0:512]
                    nc.scalar.activation(
                        rbf[:, 0:512], diff[:, 0:512], AF.Exp, bias=ob_s
                    )
                # lhsT for e is the single-stride column comb s*8+e (s=2p+h)
                rbf_e = rbf.rearrange("p (s e) -> p e s", s=128, e=8)

                stage = stpool.tile([128, 2048], f16, tag="stage")
                stv = stage.rearrange(
                    "p (g ep e2 d) -> p ep e2 g d", g=4, ep=4, e2=2, d=64
                )
                # evac engine rotation (Pool cannot touch PSUM on TRN2):
                # DVE-heavy split with ACT taking ~1/3
                if t % 3 == 2:
                    evac = (nc.vector, nc.scalar, nc.vector, nc.scalar)
                else:
                    evac = (nc.vector, nc.scalar, nc.vector, nc.vector)
                for ep in range(4):
                    eps = psB.tile([128, 512], f32, tag="eps")
                    for e2 in (0, 1):
                        e = 2 * ep + e2
                        nc.tensor.matmul(
                            eps[0:npart, e2 * 256 : (e2 + 1) * 256],
                            rbf_e[:, e][:, 0:npart],
                            wcf[:],
                        )
                    eng = evac[ep]
                    if eng is nc.scalar:
                        nc.scalar.activation(
                            stv[0:npart, ep], eps[0:npart, :], AF.Copy
                        )
                    else:
                        eng.tensor_copy(stv[0:npart, ep], eps[0:npart, :])

                dstA = out3[8 * A : 8 * A + 8, 8 * A : 512, :].rearrange(
                    "(h g) (u e) d -> u h g e d", h=2, g=4, u=jcA // 8, e=8
                )
                nc.sync.dma_start(dstA, stage[0:nsA, :])
                if Bb is not None:
                    dstB = out3[8 * Bb : 8 * Bb + 8, 8 * Bb : 512, :].rearrange(
                        "(h g) (u e) d -> u h g e d", h=2, g=4, u=nsB // 2, e=8
                    )
                    nc.sync.dma_start(dstB, stage[nsA : nsA + nsB, :])

    _fix_waits(nc, mybir)
    return nc


def _host_inputs(pred_coords, W, b):
    o = np.linspace(0.0, CUTOFF, K)
    coeff = -0.5 / (o[1] - o[0]) ** 2

    x64 = pred_coords.astype(np.float64)  # [B, N, 3]
    r = (x64 * x64).sum(-1)  # [B, N]
    ones = np.ones((B, N), np.float64)
    lg = np.stack(
        [x64[:, :, 0], x64[:, :, 1], x64[:, :, 2], r, ones], axis=1
    ).astype(np.float32)  # [B, 5, N]
    rg = np.stack(
        [-2 * x64[:, :, 0], -2 * x64[:, :, 1], -2 * x64[:, :, 2], ones, r],
        axis=1,
    ).astype(np.float32)  # [B, 5, N]

    ct = np.zeros((128, CW), np.float32)

    # dm: diag-zero mask scaled by coeff, per 128-i chunk
    dm = np.full((128, 4, N), np.float32(coeff), np.float32)
    for q in range(4):
        dm[np.arange(128), q, 128 * q + np.arange(128)] = 0.0
    ct[:, C_DM : C_DM + 2048] = dm.reshape(128, 4 * N)

    # sel6: 6 stationary variants [120, 128]; member m's rows live at
    # partition (m*4 + i_sub)*5 + t, columns (i_sub, k)
    gam = (-2.0 * coeff) * o  # f64 [K]
    c_k = gam.astype(np.float16)
    d_k = (gam - c_k.astype(np.float64)).astype(np.float16)
    tvals = [
        np.ones(K, np.float32),
        np.ones(K, np.float32),
        c_k.astype(np.float32),
        d_k.astype(np.float32),
        c_k.astype(np.float32),
    ]
    sel = np.zeros((120, 6, 128), np.float32)
    for m in range(6):
        for isub in range(4):
            for tt in range(5):
                prow = (m * 4 + isub) * 5 + tt
                sel[prow, m, isub * 32 : (isub + 1) * 32] = tvals[tt]
    ct[0:120, C_SEL : C_SEL + 768] = sel.reshape(120, 768)

    # wc: block-diagonal W.T
    for g in range(4):
        ct[32 * g : 32 * (g + 1), C_WC + 64 * g : C_WC + 64 * (g + 1)] = W.T

    # ob: coeff * o_k^2 (ACT Exp bias), k = p % 32
    ct[:, C_OB] = np.tile((coeff * o * o).astype(np.float32), 4)

    cts = []
    for cidx in range(B):
        cc = ct.copy()
        cc[0:5, C_LG : C_LG + N] = lg[cidx]
        cc[0:5, C_RG : C_RG + N] = rg[cidx]
        cts.append(cc)
    return cts


def kernel(pred_coords, mask, W, b):
    from concourse.bass_utils import run_bass_kernel_spmd

    pred_coords = np.asarray(pred_coords)
    mask = np.asarray(mask)
    W = np.asarray(W)
    b = np.asarray(b).astype(np.float32)

    if "nc" not in _CACHE:
        _CACHE["nc"] = _build_program()
    nc = _CACHE["nc"]

    cts = _host_inputs(pred_coords, W, b)
    in_maps = [{"ct": cts[c]} for c in range(B)]
    import os
    tdir = os.environ.get("KTRACE_DIR") or None
    res = run_bass_kernel_spmd(
        nc, in_maps, list(range(B)), trace=TRACE, tmpdir=tdir
    )
    _CACHE["last_res"] = res

    I, J = np.tril_indices(64, k=-1)
    outs = []
    for c in range(B):
        o16 = np.array(res.results[c]["out"])  # [N, N, 64] fp16, upper tri
        v = o16.reshape(64, 8, 64, 8, 64)
        v[I, :, J] = v[J, :, I].swapaxes(1, 2)  # mirror lower block-triangle
        out = o16.astype(np.float32)
        out += b
        outs.append(out)
    out = np.stack(outs)  # [B, N, N, 64]

    if not np.all(mask == 1.0):
        adj = (mask[:, None, :] * mask[:, :, None]).astype(np.float32)
        out = out * adj[..., None]
    return out



# revision 4
# speedup vs baseline: 2.5817x; 2.5817x over previous
"""Trainium2 Bass kernel for nn_DistanceEdgeSelfCond.

Computes, for inputs pred_coords [8,512,3], mask [8,512], W [64,32], b [64]:
    d[i,j]   = ||x_i - x_j||                        (pairwise distances)
    rbf      = exp(coeff * (d - o_k)^2)             (gaussian smearing, K=32)
    edge     = rbf @ W.T + b                        ([B,512,512,64])
    out      = edge * (mask_i * mask_j)[...,None]

Sharding: data-parallel over B — one batch per NeuronCore (8 cores).

Device strategy (per core, one batch element):
  * The output is symmetric in (i,j), so the device computes only the
    upper block-triangle (8-row blocks); the host mirrors the rest.
    33 uniform iterations cover it: row-block 0 alone (64 j-chunks),
    pairs (a, 64-a) for a=1..31 (64 j-chunks combined), row-block 32
    alone (32 chunks).
  * Per 4-row half-block, ONE fp16 matmul (contraction over 120
    partitions holding {hi/lo splits of coeff*d^2 and d} x 4 rows x 5
    terms for 6 half-blocks) produces arg = coeff*(d^2 - 2*o_k*d) in
    PSUM at [128=(i_sub,k), j] layout; coeff*o_k^2 rides the ACT Exp
    bias.  fp16 products are exact in the f32 PSUM, and hi/lo splits
    carry f32-grade precision through the fp16 operands.
  * ACT Exp (with per-partition bias) -> rbf fp16, laid out as the
    edge-matmul lhsT.
  * Edge matmul vs block-diagonal W.T (fp16): 8 matmuls of 256 cols
    per iteration; out partitions = (j-octet, i-half), cols = (i_sub,d).
  * PSUM evacuation (f32 -> fp16) rotates across DVE / Pool / ACT.
  * Output written to HBM as fp16 (halves DMA-write traffic); host
    upcasts, adds the bias b, applies the mask, and mirrors the lower
    triangle.

Walrus's PE LDWEIGHTS struct carries at most ONE sync wait, so a
post-pass moves excess waits onto InstNoOp in the same engine stream.
"""

import sys

import numpy as np

for _p in ("/opt/trn_rl_repo", "/root/.axon_site/_ro/trn_rl_repo"):
    if _p not in sys.path:
        sys.path.append(_p)

B = 8
N = 512
K = 32
D = 64
CUTOFF = 10.0

# column offsets inside the merged constant tensor [128, CW] (f32)
C_LG = 0          # rows 0:5, cols 0:512
C_RG = 512        # rows 0:5, cols 512:1024
C_DM = 1024       # [128, 2048] diag-zero mask * coeff, per 128-i chunk
C_SEL = 3072      # rows 0:120, 6 x 128 cols  (sel6 stationary variants)
C_WC = 3840       # [128, 256]  block-diagonal W.T
C_OB = 4096       # [128, 1]    coeff * o_k^2 per partition (k = p % 32)
CW = 4097

NG = 22           # gather groups of 6 half-blocks (last partial: 2)

_CACHE = {}
TRACE = False  # set True (e.g. from test.py) to capture an NTFF profile


def _fix_waits(nc, mybir):
    """Enforce <=1 embedded sync wait on compute-engine instructions."""
    limited = {
        mybir.EngineType.PE,
        mybir.EngineType.DVE,
        mybir.EngineType.Activation,
        mybir.EngineType.SP,
        mybir.EngineType.Pool,
    }
    for blk in nc.m.functions[0].blocks:
        insts = blk.instructions
        i = 0
        while i < len(insts):
            inst = insts[i]
            si = inst.sync_info
            if (
                inst.engine in limited
                and si is not None
                and si.on_wait
                and len(si.on_wait) > 1
            ):
                waits = list(si.on_wait)
                excess, keep = waits[:-1], waits[-1:]
                for w in excess:
                    nop = mybir.InstNoOp(
                        name=nc.get_next_instruction_name(),
                        sync_info=mybir.SyncInfo(on_wait=[w], on_update=[]),
                        bass_nofuse=True,
                        engine=inst.engine,
                    )
                    nc.register_instruction(nop)
                    insts.insert(i, nop)
                    i += 1
                si.on_wait = keep
            i += 1


def _iters():
    """(A, B) row-block pairs: 33 iterations covering the block triangle."""
    its = [(0, None)]
    its += [(a, 64 - a) for a in range(1, 32)]
    its.append((32, None))
    return its


def _build_program():
    import concourse.bass as bass
    import concourse.tile as tile
    from concourse import mybir

    f32 = mybir.dt.float32
    f16 = mybir.dt.float16
    AF = mybir.ActivationFunctionType

    o = np.linspace(0.0, CUTOFF, K)
    coeff = float(-0.5 / (o[1] - o[0]) ** 2)

    nc = bass.Bass("TRN2", target_bir_lowering=False, debug=False)

    ct_d = nc.dram_tensor("ct", [128, CW], f32, kind="ExternalInput")
    # staging layout: one [128, 2048] fp16 tile per iteration, written as a
    # single partition-major fully-contiguous DMA (stripes across all 16
    # SDMA engines at ~307 GB/s; the scattered per-(i,j) layout drained at
    # single-engine rate). Host decodes (u, h, g, e, d) -> (i, j, d).
    out_d = nc.dram_tensor("out", [33, 128, 2048], f16, kind="ExternalOutput")
    stg = out_d.ap()

    with tile.TileContext(nc) as tc:
        with (
            tc.tile_pool(name="consts", bufs=1) as consts,
            tc.tile_pool(name="dstore", bufs=1) as dstore,
            tc.tile_pool(name="work", bufs=2) as work,
            tc.tile_pool(name="rbfp", bufs=3) as rbfp,
            tc.tile_pool(name="stpool", bufs=4) as stpool,
            tc.tile_pool(name="psA", bufs=2, space=bass.MemorySpace.PSUM) as psA,
            tc.tile_pool(name="psB", bufs=4, space=bass.MemorySpace.PSUM) as psB,
        ):
            ct_s = consts.tile([128, CW], f32, tag="ct")
            ap = ct_d.ap()
            # phase-1 deps first, bulk behind
            nc.sync.dma_start(ct_s[0:5, 0:1024], ap[0:5, 0:1024])
            nc.sync.dma_start(ct_s[:, C_DM : C_DM + 2048], ap[:, C_DM : C_DM + 2048])
            nc.sync.dma_start(
                ct_s[0:120, C_SEL : C_SEL + 768], ap[0:120, C_SEL : C_SEL + 768]
            )
            nc.sync.dma_start(
                ct_s[:, C_WC : C_WC + 257], ap[:, C_WC : C_WC + 257]
            )

            ob_s = ct_s[:, C_OB : C_OB + 1]

            # fp16 casts of matmul constants
            sel6f = consts.tile([120, 6 * 128], f16, tag="sel6f")
            nc.vector.tensor_copy(sel6f[:], ct_s[0:120, C_SEL : C_SEL + 768])
            wcf = consts.tile([128, 256], f16, tag="wcf")
            nc.vector.tensor_copy(wcf[:], ct_s[:, C_WC : C_WC + 256])

            # X5 [128, (t=5, q=4, j=512)]: per-i-row fp16 splits
            #   t0 = hi(coeff*d^2), t1 = lo, t2 = t3 = hi(d), t4 = lo(d)
            X5 = dstore.tile([128, 5 * 2048], f16, tag="X5")

            for q in range(4):
                g_ps = psB.tile([128, N], f32, tag="eps")
                nc.tensor.matmul(
                    g_ps[:],
                    ct_s[0:5, C_LG + q * 128 : C_LG + (q + 1) * 128],
                    ct_s[0:5, C_RG : C_RG + N],
                )
                draw = work.tile([128, N], f32, tag="draw")
                # a1 = relu(d^2) * coeff, diagonal zeroed (dm carries coeff)
                nc.vector.scalar_tensor_tensor(
                    draw[:],
                    g_ps[:],
                    0.0,
                    ct_s[:, C_DM + q * N : C_DM + (q + 1) * N],
                    mybir.AluOpType.max,
                    mybir.AluOpType.mult,
                )
                qs = q * N
                nc.gpsimd.tensor_copy(X5[:, 0 * 2048 + qs : 0 * 2048 + qs + N], draw[:])
                nc.gpsimd.tensor_sub(
                    X5[:, 1 * 2048 + qs : 1 * 2048 + qs + N],
                    draw[:],
                    X5[:, 0 * 2048 + qs : 0 * 2048 + qs + N],
                )
                dfull = work.tile([128, N], f32, tag="dfull")
                nc.scalar.activation(
                    dfull[:], draw[:], AF.Sqrt, scale=float(1.0 / coeff)
                )
                nc.gpsimd.tensor_copy(X5[:, 2 * 2048 + qs : 2 * 2048 + qs + N], dfull[:])
                nc.gpsimd.tensor_copy(X5[:, 3 * 2048 + qs : 3 * 2048 + qs + N], dfull[:])
                nc.gpsimd.tensor_sub(
                    X5[:, 4 * 2048 + qs : 4 * 2048 + qs + N],
                    dfull[:],
                    X5[:, 2 * 2048 + qs : 2 * 2048 + qs + N],
                )

            # movb [120, NG*512] fp16: per 6-half-block group G, partition
            # (m*4 + i_sub)*5 + t holds term t of i-row 24G + 4m + i_sub.
            movb = dstore.tile([120, NG * 512], f16, tag="movb")
            # last group holds only 2 half-blocks (40 rows); zero its chunk
            # first (gather overwrites rows 0:40) so inactive-row garbage
            # can't turn 0-cell products into NaN
            nc.vector.memset(movb[0:120, 512 * (NG - 1) : 512 * NG], 0.0)

            def emit_gather(G):
                nmem = min(6, 128 - 6 * G)  # half-blocks in this group
                r0 = 24 * G  # first global i-row
                r1 = r0 + 4 * nmem
                # split into per-q runs (q = i//128)
                s = r0
                while s < r1:
                    q = s // 128
                    e = min(r1, (q + 1) * 128)
                    p0, cnt = s % 128, e - s
                    src = X5[p0 : p0 + cnt, :].rearrange(
                        "r (t q j) -> r t q j", t=5, q=4
                    )[:, :, q, :]
                    # dst partition (r*5 + t) is r-major: flat [5*cnt, 512]
                    # enumerates (r, t, j) in the same order as src
                    dst = movb[
                        5 * (s - r0) : 5 * (s - r0) + 5 * cnt,
                        512 * G : 512 * (G + 1),
                    ]
                    nc.sync.dma_start(dst, src)
                    s = e

            # gathers ordered by first use (iterations walk both ends)
            order, seen = [], set()
            for (A, Bb) in _iters():
                hbs = [2 * A, 2 * A + 1] + ([2 * Bb, 2 * Bb + 1] if Bb else [])
                for hb in hbs:
                    G = hb // 6
                    if G not in seen:
                        seen.add(G)
                        order.append(G)
            for G in order:
                emit_gather(G)

            its = _iters()
            diff_tiles = {}

            def emit_bcast(t):
                # diff col = p*16 + h*8 + e (p = j-octet in the A|B concat,
                # h = i-half, e = j%8): slot s = 2p+h is the edge-matmul out
                # partition, so A and B land on contiguous partition runs
                # while each DMA dst run is a full (e d) = 1 KiB.
                A, Bb = its[t]
                jcA = 512 - 8 * A
                diff = psA.tile([128, 1024], f32, tag="diff")
                dv = diff.rearrange("p (u h e) -> p h u e", u=64, h=2, e=8)
                for h in (0, 1):
                    hbA = 2 * A + h
                    GA, mA = hbA // 6, hbA % 6
                    selA = sel6f[:, mA * 128 : (mA + 1) * 128]
                    movA = movb[:, 512 * GA + 8 * A : 512 * GA + 512]
                    # a matmul whose strided PSUM out crosses the 2 KiB bank
                    # boundary gets its second-bank tail clobbered by the
                    # next matmul in that bank — split at octet 32
                    nA = jcA // 8
                    nc.tensor.matmul(
                        dv[:, h, 0 : min(nA, 32)], selA, movA[:, 0 : min(jcA, 256)]
                    )
                    if nA > 32:
                        nc.tensor.matmul(
                            dv[:, h, 32:nA], selA, movA[:, 256:jcA]
                        )
                    if Bb is not None:
                        hbB = 2 * Bb + h
                        GB, mB = hbB // 6, hbB % 6
                        nc.tensor.matmul(
                            dv[:, h, nA:64],
                            sel6f[:, mB * 128 : (mB + 1) * 128],
                            movb[:, 512 * GB + 8 * Bb : 512 * GB + 512],
                        )
                diff_tiles[t] = diff

            LOOKAHEAD = 1
            for t in range(LOOKAHEAD):
                emit_bcast(t)

            for t in range(len(its)):
                A, Bb = its[t]
                jcA = 512 - 8 * A
                nsA = jcA // 4           # A slots (4 j-pixels each)
                nsB = (512 - 8 * Bb) // 4 if Bb is not None else 0
                npart = nsA + nsB        # 128 except final half iteration

                if t + LOOKAHEAD < len(its):
                    emit_bcast(t + LOOKAHEAD)
                diff = diff_tiles.pop(t)

                rbf = rbfp.tile([128, 1024], f16, tag="rbf")
                if npart == 128:
                    nc.scalar.activation(rbf[:], diff[:], AF.Exp, bias=ob_s)
                else:
                    # half iteration: used cols are the contiguous [0:512]
                    nc.scalar.activation(
                        rbf[:, 0:512], diff[:, 0:512], AF.Exp, bias=ob_s
                    )
                # lhsT for e is the single-stride column comb s*8+e (s=2p+h)
                rbf_e = rbf.rearrange("p (s e) -> p e s", s=128, e=8)

                stage = stpool.tile([128, 2048], f16, tag="stage")
                stv = stage.rearrange(
                    "p (g ep e2 d) -> p ep e2 g d", g=4, ep=4, e2=2, d=64
                )
                # evac engine rotation (Pool cannot touch PSUM on TRN2):
                # DVE-heavy split with ACT taking ~1/3
                if t % 3 == 2:
                    evac = (nc.vector, nc.scalar, nc.vector, nc.scalar)
                else:
                    evac = (nc.vector, nc.scalar, nc.vector, nc.vector)
                for ep in range(4):
                    eps = psB.tile([128, 512], f32, tag="eps")
                    for e2 in (0, 1):
                        e = 2 * ep + e2
                        nc.tensor.matmul(
                            eps[0:npart, e2 * 256 : (e2 + 1) * 256],
                            rbf_e[:, e][:, 0:npart],
                            wcf[:],
                        )
                    eng = evac[ep]
                    if eng is nc.scalar:
                        nc.scalar.activation(
                            stv[0:npart, ep], eps[0:npart, :], AF.Copy
                        )
                    else:
                        eng.tensor_copy(stv[0:npart, ep], eps[0:npart, :])

                nc.sync.dma_start(stg[t, 0:npart, :], stage[0:npart, :])

    _fix_waits(nc, mybir)
    return nc


def _host_inputs(pred_coords, W, b):
    o = np.linspace(0.0, CUTOFF, K)
    coeff = -0.5 / (o[1] - o[0]) ** 2

    x64 = pred_coords.astype(np.float64)  # [B, N, 3]
    r = (x64 * x64).sum(-1)  # [B, N]
    ones = np.ones((B, N), np.float64)
    lg = np.stack(
        [x64[:, :, 0], x64[:, :, 1], x64[:, :, 2], r, ones], axis=1
    ).astype(np.float32)  # [B, 5, N]
    rg = np.stack(
        [-2 * x64[:, :, 0], -2 * x64[:, :, 1], -2 * x64[:, :, 2], ones, r],
        axis=1,
    ).astype(np.float32)  # [B, 5, N]

    ct = np.zeros((128, CW), np.float32)

    # dm: diag-zero mask scaled by coeff, per 128-i chunk
    dm = np.full((128, 4, N), np.float32(coeff), np.float32)
    for q in range(4):
        dm[np.arange(128), q, 128 * q + np.arange(128)] = 0.0
    ct[:, C_DM : C_DM + 2048] = dm.reshape(128, 4 * N)

    # sel6: 6 stationary variants [120, 128]; member m's rows live at
    # partition (m*4 + i_sub)*5 + t, columns (i_sub, k)
    gam = (-2.0 * coeff) * o  # f64 [K]
    c_k = gam.astype(np.float16)
    d_k = (gam - c_k.astype(np.float64)).astype(np.float16)
    tvals = [
        np.ones(K, np.float32),
        np.ones(K, np.float32),
        c_k.astype(np.float32),
        d_k.astype(np.float32),
        c_k.astype(np.float32),
    ]
    sel = np.zeros((120, 6, 128), np.float32)
    for m in range(6):
        for isub in range(4):
            for tt in range(5):
                prow = (m * 4 + isub) * 5 + tt
                sel[prow, m, isub * 32 : (isub + 1) * 32] = tvals[tt]
    ct[0:120, C_SEL : C_SEL + 768] = sel.reshape(120, 768)

    # wc: block-diagonal W.T
    for g in range(4):
        ct[32 * g : 32 * (g + 1), C_WC + 64 * g : C_WC + 64 * (g + 1)] = W.T

    # ob: coeff * o_k^2 (ACT Exp bias), k = p % 32
    ct[:, C_OB] = np.tile((coeff * o * o).astype(np.float32), 4)

    cts = []
    for cidx in range(B):
        cc = ct.copy()
        cc[0:5, C_LG : C_LG + N] = lg[cidx]
        cc[0:5, C_RG : C_RG + N] = rg[cidx]
        cts.append(cc)
    return cts


def kernel(pred_coords, mask, W, b):
    from concourse.bass_utils import run_bass_kernel_spmd

    pred_coords = np.asarray(pred_coords)
    mask = np.asarray(mask)
    W = np.asarray(W)
    b = np.asarray(b).astype(np.float32)

    if "nc" not in _CACHE:
        _CACHE["nc"] = _build_program()
    nc = _CACHE["nc"]

    cts = _host_inputs(pred_coords, W, b)
    in_maps = [{"ct": cts[c]} for c in range(B)]
    import os
    tdir = os.environ.get("KTRACE_DIR") or None
    res = run_bass_kernel_spmd(
        nc, in_maps, list(range(B)), trace=TRACE, tmpdir=tdir
    )
    _CACHE["last_res"] = res

    I, J = np.tril_indices(64, k=-1)
    its = _iters()
    outs = []
    for c in range(B):
        S = np.array(res.results[c]["out"])  # [33, 128, 2048] fp16 staging
        o16 = np.empty((N, N, D), np.float16)
        for t, (A, Bb) in enumerate(its):
            nA = 64 - A
            vA = S[t, 0 : 2 * nA, :].reshape(nA, 2, 4, 8, D)  # u h g e d
            o16[8 * A : 8 * A + 8, 8 * A : 512, :] = (
                vA.transpose(1, 2, 0, 3, 4).reshape(8, 8 * nA, D)
            )
            if Bb is not None:
                nB = A
                vB = S[t, 2 * nA : 2 * nA + 2 * nB, :].reshape(nB, 2, 4, 8, D)
                o16[8 * Bb : 8 * Bb + 8, 8 * Bb : 512, :] = (
                    vB.transpose(1, 2, 0, 3, 4).reshape(8, 8 * nB, D)
                )
        v = o16.reshape(64, 8, 64, 8, 64)
        v[I, :, J] = v[J, :, I].swapaxes(1, 2)  # mirror lower block-triangle
        out = o16.astype(np.float32)
        out += b
        outs.append(out)
    out = np.stack(outs)  # [B, N, N, 64]

    if not np.all(mask == 1.0):
        adj = (mask[:, None, :] * mask[:, :, None]).astype(np.float32)
        out = out * adj[..., None]
    return out



# revision 7
# speedup vs baseline: 2.5907x; 1.0035x over previous
"""Trainium2 Bass kernel for nn_DistanceEdgeSelfCond.

Computes, for inputs pred_coords [8,512,3], mask [8,512], W [64,32], b [64]:
    d[i,j]   = ||x_i - x_j||                        (pairwise distances)
    rbf      = exp(coeff * (d - o_k)^2)             (gaussian smearing, K=32)
    edge     = rbf @ W.T + b                        ([B,512,512,64])
    out      = edge * (mask_i * mask_j)[...,None]

Sharding: data-parallel over B — one batch per NeuronCore (8 cores).

Device strategy (per core, one batch element):
  * The output is symmetric in (i,j), so the device computes only the
    upper block-triangle (8-row blocks); the host mirrors the rest.
    33 uniform iterations cover it: row-block 0 alone (64 j-chunks),
    pairs (a, 64-a) for a=1..31 (64 j-chunks combined), row-block 32
    alone (32 chunks).
  * Per 4-row half-block, ONE fp16 matmul (contraction over 120
    partitions holding {hi/lo splits of coeff*d^2 and d} x 4 rows x 5
    terms for 6 half-blocks) produces arg = coeff*(d^2 - 2*o_k*d) in
    PSUM at [128=(i_sub,k), j] layout; coeff*o_k^2 rides the ACT Exp
    bias.  fp16 products are exact in the f32 PSUM, and hi/lo splits
    carry f32-grade precision through the fp16 operands.
  * ACT Exp (with per-partition bias) -> rbf fp16, laid out as the
    edge-matmul lhsT.
  * Edge matmul vs block-diagonal W.T (fp16): 8 matmuls of 256 cols
    per iteration; out partitions = (j-octet, i-half), cols = (i_sub,d).
  * PSUM evacuation (f32 -> fp16) rotates across DVE / Pool / ACT.
  * Output written to HBM as fp16 (halves DMA-write traffic); host
    upcasts, adds the bias b, applies the mask, and mirrors the lower
    triangle.

Walrus's PE LDWEIGHTS struct carries at most ONE sync wait, so a
post-pass moves excess waits onto InstNoOp in the same engine stream.
"""

import sys

import numpy as np

for _p in ("/opt/trn_rl_repo", "/root/.axon_site/_ro/trn_rl_repo"):
    if _p not in sys.path:
        sys.path.append(_p)

B = 8
N = 512
K = 32
D = 64
CUTOFF = 10.0

# column offsets inside the merged constant tensor [128, CW] (f32)
C_LG = 0          # rows 0:5, cols 0:512
C_RG = 512        # rows 0:5, cols 512:1024
C_DM = 1024       # [128, 2048] diag-zero mask * coeff, per 128-i chunk
C_SEL = 3072      # rows 0:120, 6 x 128 cols  (sel6 stationary variants)
C_WC = 3840       # [128, 256]  block-diagonal W.T
C_OB = 4096       # [128, 1]    coeff * o_k^2 per partition (k = p % 32)
CW = 4097

NG = 22           # gather groups of 6 half-blocks (last partial: 2)

_CACHE = {}
TRACE = False  # set True (e.g. from test.py) to capture an NTFF profile


def _fix_waits(nc, mybir):
    """Enforce <=1 embedded sync wait on compute-engine instructions."""
    limited = {
        mybir.EngineType.PE,
        mybir.EngineType.DVE,
        mybir.EngineType.Activation,
        mybir.EngineType.SP,
        mybir.EngineType.Pool,
    }
    for blk in nc.m.functions[0].blocks:
        insts = blk.instructions
        i = 0
        while i < len(insts):
            inst = insts[i]
            si = inst.sync_info
            if (
                inst.engine in limited
                and si is not None
                and si.on_wait
                and len(si.on_wait) > 1
            ):
                waits = list(si.on_wait)
                excess, keep = waits[:-1], waits[-1:]
                for w in excess:
                    nop = mybir.InstNoOp(
                        name=nc.get_next_instruction_name(),
                        sync_info=mybir.SyncInfo(on_wait=[w], on_update=[]),
                        bass_nofuse=True,
                        engine=inst.engine,
                    )
                    nc.register_instruction(nop)
                    insts.insert(i, nop)
                    i += 1
                si.on_wait = keep
            i += 1


def _iters():
    """(A, B) row-block pairs: 33 iterations covering the block triangle."""
    its = [(0, None)]
    its += [(a, 64 - a) for a in range(1, 32)]
    its.append((32, None))
    return its


def _build_program():
    import concourse.bass as bass
    import concourse.tile as tile
    from concourse import mybir

    f32 = mybir.dt.float32
    f16 = mybir.dt.float16
    AF = mybir.ActivationFunctionType

    o = np.linspace(0.0, CUTOFF, K)
    coeff = float(-0.5 / (o[1] - o[0]) ** 2)

    nc = bass.Bass("TRN2", target_bir_lowering=False, debug=False)

    ct_d = nc.dram_tensor("ct", [128, CW], f32, kind="ExternalInput")
    # staging layout: one [128, 2048] fp16 tile per iteration, written as a
    # single partition-major fully-contiguous DMA (stripes across all 16
    # SDMA engines at ~307 GB/s; the scattered per-(i,j) layout drained at
    # single-engine rate). Host decodes (u, h, g, e, d) -> (i, j, d).
    out_d = nc.dram_tensor("out", [33, 128, 2048], f16, kind="ExternalOutput")
    stg = out_d.ap()

    with tile.TileContext(nc) as tc:
        with (
            tc.tile_pool(name="consts", bufs=1) as consts,
            tc.tile_pool(name="dstore", bufs=1) as dstore,
            tc.tile_pool(name="work", bufs=2) as work,
            tc.tile_pool(name="rbfp", bufs=3) as rbfp,
            tc.tile_pool(name="stpool", bufs=4) as stpool,
            tc.tile_pool(name="psA", bufs=2, space=bass.MemorySpace.PSUM) as psA,
            tc.tile_pool(name="psB", bufs=4, space=bass.MemorySpace.PSUM) as psB,
        ):
            ct_s = consts.tile([128, CW], f32, tag="ct")
            ap = ct_d.ap()
            # phase-1 deps first, bulk behind
            nc.sync.dma_start(ct_s[0:5, 0:1024], ap[0:5, 0:1024])
            nc.sync.dma_start(ct_s[:, C_DM : C_DM + 2048], ap[:, C_DM : C_DM + 2048])
            nc.sync.dma_start(
                ct_s[0:120, C_SEL : C_SEL + 768], ap[0:120, C_SEL : C_SEL + 768]
            )
            nc.sync.dma_start(
                ct_s[:, C_WC : C_WC + 257], ap[:, C_WC : C_WC + 257]
            )

            ob_s = ct_s[:, C_OB : C_OB + 1]

            # fp16 casts of matmul constants
            sel6f = consts.tile([120, 6 * 128], f16, tag="sel6f")
            nc.vector.tensor_copy(sel6f[:], ct_s[0:120, C_SEL : C_SEL + 768])
            wcf = consts.tile([128, 256], f16, tag="wcf")
            nc.vector.tensor_copy(wcf[:], ct_s[:, C_WC : C_WC + 256])

            # X5 [128, (t=5, q=4, j=512)]: per-i-row fp16 splits
            #   t0 = hi(coeff*d^2), t1 = lo, t2 = t3 = hi(d), t4 = lo(d)
            X5 = dstore.tile([128, 5 * 2048], f16, tag="X5")

            for q in range(4):
                g_ps = psB.tile([128, N], f32, tag="eps")
                nc.tensor.matmul(
                    g_ps[:],
                    ct_s[0:5, C_LG + q * 128 : C_LG + (q + 1) * 128],
                    ct_s[0:5, C_RG : C_RG + N],
                )
                draw = work.tile([128, N], f32, tag="draw")
                # a1 = relu(d^2) * coeff, diagonal zeroed (dm carries coeff)
                nc.vector.scalar_tensor_tensor(
                    draw[:],
                    g_ps[:],
                    0.0,
                    ct_s[:, C_DM + q * N : C_DM + (q + 1) * N],
                    mybir.AluOpType.max,
                    mybir.AluOpType.mult,
                )
                qs = q * N
                # startup is latency-critical: keep these off slow GpSimd
                nc.vector.tensor_copy(X5[:, 0 * 2048 + qs : 0 * 2048 + qs + N], draw[:])
                nc.vector.tensor_sub(
                    X5[:, 1 * 2048 + qs : 1 * 2048 + qs + N],
                    draw[:],
                    X5[:, 0 * 2048 + qs : 0 * 2048 + qs + N],
                )
                dfull = work.tile([128, N], f32, tag="dfull")
                nc.scalar.activation(
                    dfull[:], draw[:], AF.Sqrt, scale=float(1.0 / coeff)
                )
                nc.scalar.activation(
                    X5[:, 2 * 2048 + qs : 2 * 2048 + qs + N], dfull[:], AF.Copy
                )
                nc.scalar.activation(
                    X5[:, 3 * 2048 + qs : 3 * 2048 + qs + N], dfull[:], AF.Copy
                )
                nc.vector.tensor_sub(
                    X5[:, 4 * 2048 + qs : 4 * 2048 + qs + N],
                    dfull[:],
                    X5[:, 2 * 2048 + qs : 2 * 2048 + qs + N],
                )

            # movb [120, NG*512] fp16: per 6-half-block group G, partition
            # (m*4 + i_sub)*5 + t holds term t of i-row 24G + 4m + i_sub.
            movb = dstore.tile([120, NG * 512], f16, tag="movb")
            # last group holds only 2 half-blocks (40 rows); zero its chunk
            # first (gather overwrites rows 0:40) so inactive-row garbage
            # can't turn 0-cell products into NaN
            nc.vector.memset(movb[0:120, 512 * (NG - 1) : 512 * NG], 0.0)

            def emit_gather(G):
                nmem = min(6, 128 - 6 * G)  # half-blocks in this group
                r0 = 24 * G  # first global i-row
                r1 = r0 + 4 * nmem
                # split into per-q runs (q = i//128)
                s = r0
                while s < r1:
                    q = s // 128
                    e = min(r1, (q + 1) * 128)
                    p0, cnt = s % 128, e - s
                    src = X5[p0 : p0 + cnt, :].rearrange(
                        "r (t q j) -> r t q j", t=5, q=4
                    )[:, :, q, :]
                    # dst partition (r*5 + t) is r-major: flat [5*cnt, 512]
                    # enumerates (r, t, j) in the same order as src
                    dst = movb[
                        5 * (s - r0) : 5 * (s - r0) + 5 * cnt,
                        512 * G : 512 * (G + 1),
                    ]
                    # SWDGE queue: keeps the sync HWDGE queue free for the
                    # staging output writes (no head-of-line blocking)
                    nc.gpsimd.dma_start(dst, src)
                    s = e

            # gathers ordered by first use (iterations walk both ends)
            order, seen = [], set()
            for (A, Bb) in _iters():
                hbs = [2 * A, 2 * A + 1] + ([2 * Bb, 2 * Bb + 1] if Bb else [])
                for hb in hbs:
                    G = hb // 6
                    if G not in seen:
                        seen.add(G)
                        order.append(G)
            for G in order:
                emit_gather(G)

            its = _iters()
            diff_tiles = {}

            def emit_bcast(t):
                # diff col = p*16 + h*8 + e (p = j-octet in the A|B concat,
                # h = i-half, e = j%8): slot s = 2p+h is the edge-matmul out
                # partition, so A and B land on contiguous partition runs
                # while each DMA dst run is a full (e d) = 1 KiB.
                A, Bb = its[t]
                jcA = 512 - 8 * A
                diff = psA.tile([128, 1024], f32, tag="diff")
                dv = diff.rearrange("p (u h e) -> p h u e", u=64, h=2, e=8)
                for h in (0, 1):
                    hbA = 2 * A + h
                    GA, mA = hbA // 6, hbA % 6
                    selA = sel6f[:, mA * 128 : (mA + 1) * 128]
                    movA = movb[:, 512 * GA + 8 * A : 512 * GA + 512]
                    # a matmul whose strided PSUM out crosses the 2 KiB bank
                    # boundary gets its second-bank tail clobbered by the
                    # next matmul in that bank — split at octet 32
                    nA = jcA // 8
                    nc.tensor.matmul(
                        dv[:, h, 0 : min(nA, 32)], selA, movA[:, 0 : min(jcA, 256)]
                    )
                    if nA > 32:
                        nc.tensor.matmul(
                            dv[:, h, 32:nA], selA, movA[:, 256:jcA]
                        )
                    if Bb is not None:
                        hbB = 2 * Bb + h
                        GB, mB = hbB // 6, hbB % 6
                        nc.tensor.matmul(
                            dv[:, h, nA:64],
                            sel6f[:, mB * 128 : (mB + 1) * 128],
                            movb[:, 512 * GB + 8 * Bb : 512 * GB + 512],
                        )
                diff_tiles[t] = diff

            LOOKAHEAD = 1
            for t in range(LOOKAHEAD):
                emit_bcast(t)

            for t in range(len(its)):
                A, Bb = its[t]
                jcA = 512 - 8 * A
                nsA = jcA // 4           # A slots (4 j-pixels each)
                nsB = (512 - 8 * Bb) // 4 if Bb is not None else 0
                npart = nsA + nsB        # 128 except final half iteration

                if t + LOOKAHEAD < len(its):
                    emit_bcast(t + LOOKAHEAD)
                diff = diff_tiles.pop(t)

                rbf = rbfp.tile([128, 1024], f16, tag="rbf")
                if npart == 128:
                    nc.scalar.activation(rbf[:], diff[:], AF.Exp, bias=ob_s)
                else:
                    # half iteration: used cols are the contiguous [0:512]
                    nc.scalar.activation(
                        rbf[:, 0:512], diff[:, 0:512], AF.Exp, bias=ob_s
                    )
                # lhsT for e is the single-stride column comb s*8+e (s=2p+h)
                rbf_e = rbf.rearrange("p (s e) -> p e s", s=128, e=8)

                stage = stpool.tile([128, 2048], f16, tag="stage")
                stv = stage.rearrange(
                    "p (g ep e2 d) -> p ep e2 g d", g=4, ep=4, e2=2, d=64
                )
                # evac engine rotation (Pool cannot touch PSUM on TRN2):
                # ACT carries the Exp already, so give it only 1/4 of evacs
                evac = (nc.vector, nc.scalar, nc.vector, nc.vector)
                for ep in range(4):
                    eps = psB.tile([128, 512], f32, tag="eps")
                    for e2 in (0, 1):
                        e = 2 * ep + e2
                        nc.tensor.matmul(
                            eps[0:npart, e2 * 256 : (e2 + 1) * 256],
                            rbf_e[:, e][:, 0:npart],
                            wcf[:],
                        )
                    eng = evac[ep]
                    if eng is nc.scalar:
                        nc.scalar.activation(
                            stv[0:npart, ep], eps[0:npart, :], AF.Copy
                        )
                    else:
                        eng.tensor_copy(stv[0:npart, ep], eps[0:npart, :])

                nc.sync.dma_start(stg[t, 0:npart, :], stage[0:npart, :])

    _fix_waits(nc, mybir)
    return nc


def _host_inputs(pred_coords, W, b):
    o = np.linspace(0.0, CUTOFF, K)
    coeff = -0.5 / (o[1] - o[0]) ** 2

    x64 = pred_coords.astype(np.float64)  # [B, N, 3]
    r = (x64 * x64).sum(-1)  # [B, N]
    ones = np.ones((B, N), np.float64)
    lg = np.stack(
        [x64[:, :, 0], x64[:, :, 1], x64[:, :, 2], r, ones], axis=1
    ).astype(np.float32)  # [B, 5, N]
    rg = np.stack(
        [-2 * x64[:, :, 0], -2 * x64[:, :, 1], -2 * x64[:, :, 2], ones, r],
        axis=1,
    ).astype(np.float32)  # [B, 5, N]

    ct = np.zeros((128, CW), np.float32)

    # dm: diag-zero mask scaled by coeff, per 128-i chunk
    dm = np.full((128, 4, N), np.float32(coeff), np.float32)
    for q in range(4):
        dm[np.arange(128), q, 128 * q + np.arange(128)] = 0.0
    ct[:, C_DM : C_DM + 2048] = dm.reshape(128, 4 * N)

    # sel6: 6 stationary variants [120, 128]; member m's rows live at
    # partition (m*4 + i_sub)*5 + t, columns (i_sub, k)
    gam = (-2.0 * coeff) * o  # f64 [K]
    c_k = gam.astype(np.float16)
    d_k = (gam - c_k.astype(np.float64)).astype(np.float16)
    tvals = [
        np.ones(K, np.float32),
        np.ones(K, np.float32),
        c_k.astype(np.float32),
        d_k.astype(np.float32),
        c_k.astype(np.float32),
    ]
    sel = np.zeros((120, 6, 128), np.float32)
    for m in range(6):
        for isub in range(4):
            for tt in range(5):
                prow = (m * 4 + isub) * 5 + tt
                sel[prow, m, isub * 32 : (isub + 1) * 32] = tvals[tt]
    ct[0:120, C_SEL : C_SEL + 768] = sel.reshape(120, 768)

    # wc: block-diagonal W.T
    for g in range(4):
        ct[32 * g : 32 * (g + 1), C_WC + 64 * g : C_WC + 64 * (g + 1)] = W.T

    # ob: coeff * o_k^2 (ACT Exp bias), k = p % 32
    ct[:, C_OB] = np.tile((coeff * o * o).astype(np.float32), 4)

    cts = []
    for cidx in range(B):
        cc = ct.copy()
        cc[0:5, C_LG : C_LG + N] = lg[cidx]
        cc[0:5, C_RG : C_RG + N] = rg[cidx]
        cts.append(cc)
    return cts


def kernel(pred_coords, mask, W, b):
    from concourse.bass_utils import run_bass_kernel_spmd

    pred_coords = np.asarray(pred_coords)
    mask = np.asarray(mask)
    W = np.asarray(W)
    b = np.asarray(b).astype(np.float32)

    if "nc" not in _CACHE:
        _CACHE["nc"] = _build_program()
    nc = _CACHE["nc"]

    cts = _host_inputs(pred_coords, W, b)
    in_maps = [{"ct": cts[c]} for c in range(B)]
    import os
    tdir = os.environ.get("KTRACE_DIR") or None
    res = run_bass_kernel_spmd(
        nc, in_maps, list(range(B)), trace=TRACE, tmpdir=tdir
    )
    _CACHE["last_res"] = res

    I, J = np.tril_indices(64, k=-1)
    its = _iters()
    outs = []
    for c in range(B):
        S = np.array(res.results[c]["out"])  # [33, 128, 2048] fp16 staging
        o16 = np.empty((N, N, D), np.float16)
        for t, (A, Bb) in enumerate(its):
            nA = 64 - A
            vA = S[t, 0 : 2 * nA, :].reshape(nA, 2, 4, 8, D)  # u h g e d
            o16[8 * A : 8 * A + 8, 8 * A : 512, :] = (
                vA.transpose(1, 2, 0, 3, 4).reshape(8, 8 * nA, D)
            )
            if Bb is not None:
                nB = A
                vB = S[t, 2 * nA : 2 * nA + 2 * nB, :].reshape(nB, 2, 4, 8, D)
                o16[8 * Bb : 8 * Bb + 8, 8 * Bb : 512, :] = (
                    vB.transpose(1, 2, 0, 3, 4).reshape(8, 8 * nB, D)
                )
        v = o16.reshape(64, 8, 64, 8, 64)
        v[I, :, J] = v[J, :, I].swapaxes(1, 2)  # mirror lower block-triangle
        out = o16.astype(np.float32)
        out += b
        outs.append(out)
    out = np.stack(outs)  # [B, N, N, 64]

    if not np.all(mask == 1.0):
        adj = (mask[:, None, :] * mask[:, :, None]).astype(np.float32)
        out = out * adj[..., None]
    return out



# revision 15
# speedup vs baseline: 3.2663x; 1.2608x over previous
"""Trainium2 Bass kernel for nn_DistanceEdgeSelfCond.

Computes, for inputs pred_coords [8,512,3], mask [8,512], W [64,32], b [64]:
    d[i,j]   = ||x_i - x_j||                        (pairwise distances)
    rbf      = exp(coeff * (d - o_k)^2)             (gaussian smearing, K=32)
    edge     = rbf @ W.T + b                        ([B,512,512,64])
    out      = edge * (mask_i * mask_j)[...,None]

Sharding: data-parallel over B — one batch per NeuronCore (8 cores).

Device strategy (per core, one batch element):
  * The output is symmetric in (i,j), so the device computes only the
    upper block-triangle (8-row blocks); the host mirrors the rest.
    33 uniform iterations cover it: row-block 0 alone (64 j-chunks),
    pairs (a, 64-a) for a=1..31 (64 j-chunks combined), row-block 32
    alone (32 chunks).
  * Per 4-row half-block, ONE fp16 matmul (contraction over 120
    partitions holding {hi/lo splits of coeff*d^2 and d} x 4 rows x 5
    terms for 6 half-blocks) produces arg = coeff*(d^2 - 2*o_k*d) in
    PSUM at [128=(i_sub,k), j] layout; coeff*o_k^2 rides the ACT Exp
    bias.  fp16 products are exact in the f32 PSUM, and hi/lo splits
    carry f32-grade precision through the fp16 operands.
  * ACT Exp (with per-partition bias) -> rbf fp16, laid out as the
    edge-matmul lhsT.
  * Edge matmul vs block-diagonal W.T (fp16): 8 matmuls of 256 cols
    per iteration; out partitions = (j-octet, i-half), cols = (i_sub,d).
  * PSUM evacuation (f32 -> fp16) rotates across DVE / Pool / ACT.
  * Output written to HBM as fp16 (halves DMA-write traffic); host
    upcasts, adds the bias b, applies the mask, and mirrors the lower
    triangle.

Walrus's PE LDWEIGHTS struct carries at most ONE sync wait, so a
post-pass moves excess waits onto InstNoOp in the same engine stream.
"""

import sys

import numpy as np

for _p in ("/opt/trn_rl_repo", "/root/.axon_site/_ro/trn_rl_repo"):
    if _p not in sys.path:
        sys.path.append(_p)

B = 8
N = 512
K = 32
D = 64
CUTOFF = 10.0

# column offsets inside the merged constant tensor [128, CW] (f32)
C_LG = 0          # rows 0:5, cols 0:512
C_RG = 512        # rows 0:5, cols 512:1024
C_DM = 1024       # [128, 2048] diag-zero mask * coeff, per 128-i chunk
C_SEL = 3072      # rows 0:120, 6 x 128 cols  (sel6 stationary variants)
C_WC = 3840       # [128, 256]  block-diagonal W.T
C_OB = 4096       # [128, 1]    coeff * o_k^2 per partition (k = p % 32)
CW = 4097

NG = 22           # gather groups of 6 half-blocks (last partial: 2)

_CACHE = {}
TRACE = False  # set True (e.g. from test.py) to capture an NTFF profile


def _fix_waits(nc, mybir):
    """Enforce <=1 embedded sync wait on compute-engine instructions."""
    limited = {
        mybir.EngineType.PE,
        mybir.EngineType.DVE,
        mybir.EngineType.Activation,
        mybir.EngineType.SP,
        mybir.EngineType.Pool,
    }
    for blk in nc.m.functions[0].blocks:
        insts = blk.instructions
        i = 0
        while i < len(insts):
            inst = insts[i]
            si = inst.sync_info
            if (
                inst.engine in limited
                and si is not None
                and si.on_wait
                and len(si.on_wait) > 1
            ):
                waits = list(si.on_wait)
                excess, keep = waits[:-1], waits[-1:]
                for w in excess:
                    nop = mybir.InstNoOp(
                        name=nc.get_next_instruction_name(),
                        sync_info=mybir.SyncInfo(on_wait=[w], on_update=[]),
                        bass_nofuse=True,
                        engine=inst.engine,
                    )
                    nc.register_instruction(nop)
                    insts.insert(i, nop)
                    i += 1
                si.on_wait = keep
            i += 1


def _iters():
    """(A, B) row-block pairs: 33 iterations covering the block triangle."""
    its = [(0, None)]
    its += [(a, 64 - a) for a in range(1, 32)]
    its.append((32, None))
    return its


def _build_program():
    import concourse.bass as bass
    import concourse.tile as tile
    from concourse import mybir

    f32 = mybir.dt.float32
    f16 = mybir.dt.float16
    AF = mybir.ActivationFunctionType

    o = np.linspace(0.0, CUTOFF, K)
    coeff = float(-0.5 / (o[1] - o[0]) ** 2)

    nc = bass.Bass("TRN2", target_bir_lowering=False, debug=False)

    ct_d = nc.dram_tensor("ct", [128, CW], f32, kind="ExternalInput")
    # staging layout: one [128, 2048] fp16 tile per iteration, written as a
    # single partition-major fully-contiguous DMA (stripes across all 16
    # SDMA engines at ~307 GB/s; the scattered per-(i,j) layout drained at
    # single-engine rate). Host decodes (u, h, g, e, d) -> (i, j, d).
    out_d = nc.dram_tensor("out", [33, 128, 2048], f16, kind="ExternalOutput")
    stg = out_d.ap()

    with tile.TileContext(nc) as tc:
        with (
            tc.tile_pool(name="consts", bufs=1) as consts,
            tc.tile_pool(name="dstore", bufs=1) as dstore,
            tc.tile_pool(name="work", bufs=2) as work,
            tc.tile_pool(name="rbfp", bufs=3) as rbfp,
            tc.tile_pool(name="stpool", bufs=4) as stpool,
            tc.tile_pool(name="psA", bufs=2, space=bass.MemorySpace.PSUM) as psA,
            tc.tile_pool(name="psB", bufs=4, space=bass.MemorySpace.PSUM) as psB,
        ):
            ct_s = consts.tile([128, CW], f32, tag="ct")
            ap = ct_d.ap()
            # phase-1 deps first, bulk behind
            nc.sync.dma_start(ct_s[0:15, 0:1024], ap[0:15, 0:1024])
            nc.sync.dma_start(ct_s[:, C_DM : C_DM + 2048], ap[:, C_DM : C_DM + 2048])
            nc.sync.dma_start(
                ct_s[0:120, C_SEL : C_SEL + 768], ap[0:120, C_SEL : C_SEL + 768]
            )
            nc.sync.dma_start(
                ct_s[:, C_WC : C_WC + 257], ap[:, C_WC : C_WC + 257]
            )

            ob_s = ct_s[:, C_OB : C_OB + 1]

            # fp16 casts of matmul constants
            lgrg16 = consts.tile([15, 1024], f16, tag="lgrg16")
            nc.vector.tensor_copy(lgrg16[:], ct_s[0:15, 0:1024])
            sel6f = consts.tile([120, 6 * 128], f16, tag="sel6f")
            nc.vector.tensor_copy(sel6f[:], ct_s[0:120, C_SEL : C_SEL + 768])
            wcf = consts.tile([128, 256], f16, tag="wcf")
            nc.vector.tensor_copy(wcf[:], ct_s[:, C_WC : C_WC + 256])

            # X5 [128, (t=5, q=4, j=512)]: per-i-row fp16 splits
            #   t0 = hi(coeff*d^2), t1 = lo, t2 = t3 = hi(d), t4 = lo(d)
            X5 = dstore.tile([128, 5 * 2048], f16, tag="X5")

            for q in range(4):
                g_ps = psB.tile([128, N], f32, tag="eps")
                nc.tensor.matmul(
                    g_ps[:],
                    lgrg16[:, C_LG + q * 128 : C_LG + (q + 1) * 128],
                    lgrg16[:, C_RG : C_RG + N],
                )
                draw = work.tile([128, N], f32, tag="draw")
                # a1 = relu(d^2) * coeff, diagonal zeroed (dm carries coeff)
                nc.vector.scalar_tensor_tensor(
                    draw[:],
                    g_ps[:],
                    0.0,
                    ct_s[:, C_DM + q * N : C_DM + (q + 1) * N],
                    mybir.AluOpType.max,
                    mybir.AluOpType.mult,
                )
                qs = q * N
                # startup is latency-critical: keep these off slow GpSimd
                nc.vector.tensor_copy(X5[:, 0 * 2048 + qs : 0 * 2048 + qs + N], draw[:])
                nc.vector.tensor_sub(
                    X5[:, 1 * 2048 + qs : 1 * 2048 + qs + N],
                    draw[:],
                    X5[:, 0 * 2048 + qs : 0 * 2048 + qs + N],
                )
                dfull = work.tile([128, N], f32, tag="dfull")
                nc.scalar.activation(
                    dfull[:], draw[:], AF.Sqrt, scale=float(1.0 / coeff)
                )
                nc.scalar.activation(
                    X5[:, 2 * 2048 + qs : 2 * 2048 + qs + N], dfull[:], AF.Copy
                )
                nc.scalar.activation(
                    X5[:, 3 * 2048 + qs : 3 * 2048 + qs + N], dfull[:], AF.Copy
                )
                nc.vector.tensor_sub(
                    X5[:, 4 * 2048 + qs : 4 * 2048 + qs + N],
                    dfull[:],
                    X5[:, 2 * 2048 + qs : 2 * 2048 + qs + N],
                )

            # movb [120, NG*512] fp16: per 6-half-block group G, partition
            # (m*4 + i_sub)*5 + t holds term t of i-row 24G + 4m + i_sub.
            movb = dstore.tile([120, NG * 512], f16, tag="movb")
            # last group holds only 2 half-blocks (40 rows); zero its chunk
            # first (gather overwrites rows 0:40) so inactive-row garbage
            # can't turn 0-cell products into NaN
            nc.vector.memset(movb[0:120, 512 * (NG - 1) : 512 * NG], 0.0)

            def emit_gather(G, eng):
                nmem = min(6, 128 - 6 * G)  # half-blocks in this group
                r0 = 24 * G  # first global i-row
                r1 = r0 + 4 * nmem
                # split into per-q runs (q = i//128)
                s = r0
                while s < r1:
                    q = s // 128
                    e = min(r1, (q + 1) * 128)
                    p0, cnt = s % 128, e - s
                    src = X5[p0 : p0 + cnt, :].rearrange(
                        "r (t q j) -> r t q j", t=5, q=4
                    )[:, :, q, :]
                    # dst partition (r*5 + t) is r-major: flat [5*cnt, 512]
                    # enumerates (r, t, j) in the same order as src
                    dst = movb[
                        5 * (s - r0) : 5 * (s - r0) + 5 * cnt,
                        512 * G : 512 * (G + 1),
                    ]
                    eng.dma_start(dst, src)
                    s = e

            # gathers ordered by first use (iterations walk both ends)
            order, seen = [], set()
            for (A, Bb) in _iters():
                hbs = [2 * A, 2 * A + 1] + ([2 * Bb, 2 * Bb + 1] if Bb else [])
                for hb in hbs:
                    G = hb // 6
                    if G not in seen:
                        seen.add(G)
                        order.append(G)
            # first-used groups on the fast HWDGE sync queue (issues in a
            # few hundred ns each, before the staging writes start); the
            # tail on the Pool SWDGE queue, fully off the critical engines
            for gi, G in enumerate(order):
                emit_gather(G, nc.sync if gi < 10 else nc.gpsimd)

            its = _iters()
            diff_tiles = {}

            def emit_bcast(t):
                # diff col = p*16 + h*8 + e (p = j-octet in the A|B concat,
                # h = i-half, e = j%8): slot s = 2p+h is the edge-matmul out
                # partition, so A and B land on contiguous partition runs
                # while each DMA dst run is a full (e d) = 1 KiB.
                A, Bb = its[t]
                jcA = 512 - 8 * A
                diff = psA.tile([128, 1024], f32, tag="diff")
                dv = diff.rearrange("p (u h e) -> p h u e", u=64, h=2, e=8)
                for h in (0, 1):
                    hbA = 2 * A + h
                    GA, mA = hbA // 6, hbA % 6
                    selA = sel6f[:, mA * 128 : (mA + 1) * 128]
                    movA = movb[:, 512 * GA + 8 * A : 512 * GA + 512]
                    # a matmul whose strided PSUM out crosses the 2 KiB bank
                    # boundary gets its second-bank tail clobbered by the
                    # next matmul in that bank — split at octet 32
                    nA = jcA // 8
                    nc.tensor.matmul(
                        dv[:, h, 0 : min(nA, 32)], selA, movA[:, 0 : min(jcA, 256)]
                    )
                    if nA > 32:
                        nc.tensor.matmul(
                            dv[:, h, 32:nA], selA, movA[:, 256:jcA]
                        )
                    if Bb is not None:
                        hbB = 2 * Bb + h
                        GB, mB = hbB // 6, hbB % 6
                        nc.tensor.matmul(
                            dv[:, h, nA:64],
                            sel6f[:, mB * 128 : (mB + 1) * 128],
                            movb[:, 512 * GB + 8 * Bb : 512 * GB + 512],
                        )
                diff_tiles[t] = diff

            LOOKAHEAD = 1
            for t in range(LOOKAHEAD):
                emit_bcast(t)

            for t in range(len(its)):
                A, Bb = its[t]
                jcA = 512 - 8 * A
                nsA = jcA // 4           # A slots (4 j-pixels each)
                nsB = (512 - 8 * Bb) // 4 if Bb is not None else 0
                npart = nsA + nsB        # 128 except final half iteration

                if t + LOOKAHEAD < len(its):
                    emit_bcast(t + LOOKAHEAD)
                diff = diff_tiles.pop(t)

                rbf = rbfp.tile([128, 1024], f16, tag="rbf")
                if npart == 128:
                    nc.scalar.activation(rbf[:], diff[:], AF.Exp, bias=ob_s)
                else:
                    # half iteration: used cols are the contiguous [0:512]
                    nc.scalar.activation(
                        rbf[:, 0:512], diff[:, 0:512], AF.Exp, bias=ob_s
                    )
                # lhsT for e is the single-stride column comb s*8+e (s=2p+h)
                rbf_e = rbf.rearrange("p (s e) -> p e s", s=128, e=8)

                stage = stpool.tile([128, 2048], f16, tag="stage")
                stv = stage.rearrange(
                    "p (g ep e2 d) -> p ep e2 g d", g=4, ep=4, e2=2, d=64
                )
                # evac engine rotation (Pool cannot touch PSUM on TRN2):
                # ACT carries the Exp already; ~1.25/4 of evacs lands ACT
                # and DVE near-equal (DVE copy ~0.70us, ACT evac ~0.85us)
                if t % 4 == 3:
                    evac = (nc.vector, nc.scalar, nc.vector, nc.scalar)
                else:
                    evac = (nc.vector, nc.scalar, nc.vector, nc.vector)
                for ep in range(4):
                    eps = psB.tile([128, 512], f32, tag="eps")
                    for e2 in (0, 1):
                        e = 2 * ep + e2
                        nc.tensor.matmul(
                            eps[0:npart, e2 * 256 : (e2 + 1) * 256],
                            rbf_e[:, e][:, 0:npart],
                            wcf[:],
                        )
                    eng = evac[ep]
                    if eng is nc.scalar:
                        nc.scalar.activation(
                            stv[0:npart, ep], eps[0:npart, :], AF.Copy
                        )
                    else:
                        eng.tensor_copy(stv[0:npart, ep], eps[0:npart, :])

                nc.sync.dma_start(stg[t, 0:npart, :], stage[0:npart, :])

    _fix_waits(nc, mybir)
    return nc


def _host_inputs(pred_coords, W, b):
    o = np.linspace(0.0, CUTOFF, K)
    coeff = -0.5 / (o[1] - o[0]) ** 2

    x64 = pred_coords.astype(np.float64)  # [B, N, 3]
    r = (x64 * x64).sum(-1)  # [B, N]
    ones = np.ones((B, N), np.float64)
    lg = np.stack(
        [x64[:, :, 0], x64[:, :, 1], x64[:, :, 2], r, ones], axis=1
    )  # [B, 5, N] f64
    rg = np.stack(
        [-2 * x64[:, :, 0], -2 * x64[:, :, 1], -2 * x64[:, :, 2], ones, r],
        axis=1,
    )  # [B, 5, N] f64
    # fp16 hi/lo split so the d^2 matmul runs in fp16 (exact products into
    # the f32 PSUM; the dropped lo*lo term is <= ~6e-5):
    # contraction rows 3t+{0,1,2} = (lg_hi*rg_hi, lg_hi*rg_lo, lg_lo*rg_hi)
    lgh = lg.astype(np.float16).astype(np.float64)
    lgl = lg - lgh
    rgh = rg.astype(np.float16).astype(np.float64)
    rgl = rg - rgh
    lg15 = np.empty((B, 15, N), np.float32)
    rg15 = np.empty((B, 15, N), np.float32)
    for tt in range(5):
        lg15[:, 3 * tt + 0] = lgh[:, tt]
        lg15[:, 3 * tt + 1] = lgh[:, tt]
        lg15[:, 3 * tt + 2] = lgl[:, tt].astype(np.float16)
        rg15[:, 3 * tt + 0] = rgh[:, tt]
        rg15[:, 3 * tt + 1] = rgl[:, tt].astype(np.float16)
        rg15[:, 3 * tt + 2] = rgh[:, tt]

    ct = np.zeros((128, CW), np.float32)

    # dm: diag-zero mask scaled by coeff, per 128-i chunk
    dm = np.full((128, 4, N), np.float32(coeff), np.float32)
    for q in range(4):
        dm[np.arange(128), q, 128 * q + np.arange(128)] = 0.0
    ct[:, C_DM : C_DM + 2048] = dm.reshape(128, 4 * N)

    # sel6: 6 stationary variants [120, 128]; member m's rows live at
    # partition (m*4 + i_sub)*5 + t, columns (i_sub, k)
    gam = (-2.0 * coeff) * o  # f64 [K]
    c_k = gam.astype(np.float16)
    d_k = (gam - c_k.astype(np.float64)).astype(np.float16)
    tvals = [
        np.ones(K, np.float32),
        np.ones(K, np.float32),
        c_k.astype(np.float32),
        d_k.astype(np.float32),
        c_k.astype(np.float32),
    ]
    sel = np.zeros((120, 6, 128), np.float32)
    for m in range(6):
        for isub in range(4):
            for tt in range(5):
                prow = (m * 4 + isub) * 5 + tt
                sel[prow, m, isub * 32 : (isub + 1) * 32] = tvals[tt]
    ct[0:120, C_SEL : C_SEL + 768] = sel.reshape(120, 768)

    # wc: block-diagonal W.T
    for g in range(4):
        ct[32 * g : 32 * (g + 1), C_WC + 64 * g : C_WC + 64 * (g + 1)] = W.T

    # ob: coeff * o_k^2 (ACT Exp bias), k = p % 32
    ct[:, C_OB] = np.tile((coeff * o * o).astype(np.float32), 4)

    cts = []
    for cidx in range(B):
        cc = ct.copy()
        cc[0:15, C_LG : C_LG + N] = lg15[cidx]
        cc[0:15, C_RG : C_RG + N] = rg15[cidx]
        cts.append(cc)
    return cts


def kernel(pred_coords, mask, W, b):
    from concourse.bass_utils import run_bass_kernel_spmd

    pred_coords = np.asarray(pred_coords)
    mask = np.asarray(mask)
    W = np.asarray(W)
    b = np.asarray(b).astype(np.float32)

    if "nc" not in _CACHE:
        _CACHE["nc"] = _build_program()
    nc = _CACHE["nc"]

    cts = _host_inputs(pred_coords, W, b)
    in_maps = [{"ct": cts[c]} for c in range(B)]
    import os
    tdir = os.environ.get("KTRACE_DIR") or None
    res = run_bass_kernel_spmd(
        nc, in_maps, list(range(B)), trace=TRACE, tmpdir=tdir
    )
    _CACHE["last_res"] = res

    I, J = np.tril_indices(64, k=-1)
    its = _iters()
    outs = []
    for c in range(B):
        S = np.array(res.results[c]["out"])  # [33, 128, 2048] fp16 staging
        o16 = np.empty((N, N, D), np.float16)
        for t, (A, Bb) in enumerate(its):
            nA = 64 - A
            vA = S[t, 0 : 2 * nA, :].reshape(nA, 2, 4, 8, D)  # u h g e d
            o16[8 * A : 8 * A + 8, 8 * A : 512, :] = (
                vA.transpose(1, 2, 0, 3, 4).reshape(8, 8 * nA, D)
            )
            if Bb is not None:
                nB = A
                vB = S[t, 2 * nA : 2 * nA + 2 * nB, :].reshape(nB, 2, 4, 8, D)
                o16[8 * Bb : 8 * Bb + 8, 8 * Bb : 512, :] = (
                    vB.transpose(1, 2, 0, 3, 4).reshape(8, 8 * nB, D)
                )
        v = o16.reshape(64, 8, 64, 8, 64)
        v[I, :, J] = v[J, :, I].swapaxes(1, 2)  # mirror lower block-triangle
        out = o16.astype(np.float32)
        out += b
        outs.append(out)
    out = np.stack(outs)  # [B, N, N, 64]

    if not np.all(mask == 1.0):
        adj = (mask[:, None, :] * mask[:, :, None]).astype(np.float32)
        out = out * adj[..., None]
    return out

